# revision 21
# baseline (speedup 1.0000x reference)
"""Self-attention (SAGAN-style) Bass kernel for one TRN2 chip (8 NeuronCores).

Reference computation (B=4, H=W=64, C=256, D=32, N=H*W=4096):
    xf = x.reshape(B, N, C)
    k = xf @ Wk + bk; q = xf @ Wq + bq; v = xf @ Wv + bv
    energy = q @ k^T            # [B, N, N]
    attn = softmax(energy, -1)
    feat = attn @ v
    out = gamma * (feat @ Wo + bo) + xf

Sharding: core i handles batch b=i//2, query-row half h=i%2 (2048 rows).
k/v are computed over the full 4096 rows on every core (replicated, cheap).

Host-side exact folds (no device cost):
  - bk drops out of softmax (adds a per-row constant to energy).
  - bq enters energy as (k @ bq)[m]: fold Wk@bq as an extra column of Wk,
    paired with a constant-1 row appended to q^T (energy contraction K=33).
  - v-bias: attn rows sum to 1 so attn@(v+bv) = attn@v + bv; fold
    gamma*(bv@Wo + bo) into an extra row of Wo paired with a ones row of
    feat^T; gamma scales Wo itself.

Device pipeline per core (all layouts chosen so softmax reduces along the
matmul contraction axis and no on-device transposes are needed):
  kT_aug[33,4096] = Wk_aug^T @ x^T        qT_aug[33,2048] (row 32 = 1)
  v_aug[4096,33]  (col 32 = 1)
  per m-tile: S^T[m,q] on TensorE (float32r), exp on ScalarE (PSUM->SBUF),
  PV: U^T[33,512] += v_aug[m]^T @ expS^T[m]   (row 32 = row-sums r)
  feat^T = U^T[0:32] * (1/r) (rank-1 PE broadcast of 1/r), O = feat_aug^T^T
  @ Wo_aug, out = O + x_rows.
"""
import numpy as np
from contextlib import ExitStack

import concourse.bass as bass
import concourse.bacc as bacc
import concourse.tile as tile
from concourse import mybir
from concourse import bass_utils

F32 = mybir.dt.float32
F32R = mybir.dt.float32r

B, HH, WW, C = 4, 64, 64, 256
N = HH * WW          # 4096 key/value rows
D = 32               # head dim
NCORES = 8
QSH = N // 2         # 2048 query rows per core
SBW = 512            # superblock width (q columns per S^T matmul)
NSB = QSH // SBW     # 4 superblocks
NMT = N // 128       # 32 m-tiles
ts = bass.ts


def build_graph():
    """Build and compile the per-core Bass graph (identical on all cores)."""
    nc = bacc.Bacc("TRN2", target_bir_lowering=False, debug=False)

    xT_d = nc.dram_tensor("xT", [C, N], F32R, kind="ExternalInput").ap()
    xr_d = nc.dram_tensor("xr", [QSH, C], F32, kind="ExternalInput").ap()
    wk_d = nc.dram_tensor("wk", [C, 33], F32R, kind="ExternalInput").ap()
    wq_d = nc.dram_tensor("wq", [C, D], F32R, kind="ExternalInput").ap()
    wv_d = nc.dram_tensor("wv", [C, D], F32R, kind="ExternalInput").ap()
    wo_d = nc.dram_tensor("wo", [33, C + 2], F32R, kind="ExternalInput").ap()
    out_d = nc.dram_tensor("out", [QSH, C], F32, kind="ExternalOutput").ap()

    with tile.TileContext(nc) as tc, ExitStack() as ctx:
        persist = ctx.enter_context(tc.tile_pool(name="persist", bufs=1))
        st_pool = ctx.enter_context(
            tc.tile_pool(name="stps", bufs=2, space="PSUM")
        )
        uT_pool = ctx.enter_context(
            tc.tile_pool(name="uTps", bufs=1, space="PSUM")
        )
        expp = ctx.enter_context(tc.tile_pool(name="expp", bufs=3))
        smallp = ctx.enter_context(tc.tile_pool(name="smallp", bufs=2))
        outp = ctx.enter_context(tc.tile_pool(name="outp", bufs=3))

        # ---- persistent SBUF tensors ----
        xT0 = persist.tile([128, N], F32R)   # x^T rows 0:128 (channels)
        xT1 = persist.tile([128, N], F32R)   # x^T rows 128:256
        xr_sb = persist.tile([128, 16 * C], F32)  # residual rows, tile t at cols 256t
        wk_sb = persist.tile([128, 66], F32R)
        wq_sb = persist.tile([128, 64], F32R)
        wv_sb = persist.tile([128, 64], F32R)
        wo_sb = persist.tile([33, C + 2], F32R)
        # kT2: m-tiles 0..15 in rows 0:33 (cols 128*g), m-tiles 16..31 in
        # rows 64:97 — lets S^T row-pack pairs (g, g+16) at row groups 0/64.
        kT_sb = persist.tile([128, N // 2], F32R)
        # qT2: rows 0:33 = qT_aug, rows 64:97 = duplicate (for row group 64)
        qT_sb = persist.tile([128, QSH], F32R)
        v_sb = persist.tile([128, 33 * NMT], F32R)

        # ---- input DMAs ----
        nc.sync.dma_start(wk_sb[:, 0:33], wk_d[0:128, :])
        nc.sync.dma_start(wk_sb[:, 33:66], wk_d[128:256, :])
        nc.sync.dma_start(wq_sb[:, 0:32], wq_d[0:128, :])
        nc.sync.dma_start(wq_sb[:, 32:64], wq_d[128:256, :])
        nc.sync.dma_start(wv_sb[:, 0:32], wv_d[0:128, :])
        nc.sync.dma_start(wv_sb[:, 32:64], wv_d[128:256, :])
        nc.sync.dma_start(wo_sb[:], wo_d)
        nc.sync.dma_start(xT0[:], xT_d[0:128, :])
        nc.sync.dma_start(xT1[:], xT_d[128:256, :])
        for t in range(16):
            nc.sync.dma_start(xr_sb[:, ts(t, C)], xr_d[ts(t, 128), :])

        nc.vector.memset(qT_sb[32:33, :].bitcast(F32), 1.0)
        nc.vector.memset(v_sb[:].bitcast(F32), 1.0)

        # ---- projections ----
        # qT rows 0:32 = Wq^T @ xT (own-half columns of xT are 0:QSH)
        for nt in range(QSH // SBW):
            pq = st_pool.tile([32, SBW], F32, tag="st")
            nc.tensor.matmul(pq[:], wq_sb[:, 0:32], xT0[:, ts(nt, SBW)],
                             start=True, stop=False)
            nc.tensor.matmul(pq[:], wq_sb[:, 32:64], xT1[:, ts(nt, SBW)],
                             start=False, stop=True)
            nc.vector.tensor_copy(qT_sb[0:32, ts(nt, SBW)], pq[:])
        nc.vector.tensor_copy(qT_sb[64:97, :], qT_sb[0:33, :])
        # kT_aug = Wk_aug^T @ xT over all 4096 columns
        for nt in range(N // SBW):
            pk = st_pool.tile([33, SBW], F32, tag="st")
            nc.tensor.matmul(pk[:], wk_sb[:, 0:33], xT0[:, ts(nt, SBW)],
                             start=True, stop=False)
            nc.tensor.matmul(pk[:], wk_sb[:, 33:66], xT1[:, ts(nt, SBW)],
                             start=False, stop=True)
            half = N // (2 * SBW)  # 4 n-tiles per half
            if nt < half:
                nc.vector.tensor_copy(kT_sb[0:33, ts(nt, SBW)], pk[:])
            else:
                nc.vector.tensor_copy(kT_sb[64:97, ts(nt - half, SBW)], pk[:])
        # v rows: v[m, 0:32], col 32 stays 1.0 from the memset
        for j in range(NMT):
            pv = st_pool.tile([128, 32], F32, tag="st")
            nc.tensor.matmul(pv[:], xT0[:, ts(j, 128)], wv_sb[:, 0:32],
                             start=True, stop=False)
            nc.tensor.matmul(pv[:], xT1[:, ts(j, 128)], wv_sb[:, 32:64],
                             start=False, stop=True)
            nc.vector.tensor_copy(v_sb[:, 33 * j:33 * j + 32], pv[:])

        # ---- attention: m-tile pairs (g, g+16) row/col packed on the PE ----
        # S^T pair runs concurrently in row groups 0/64; PV pair runs
        # concurrently in col groups 0/64, accumulating partial U^T sums in
        # partitions 0:33 and 64:97 of one PSUM bank (summed in the tail).
        uT = []
        for s in range(NSB):
            u = uT_pool.tile([33, SBW], F32, name=f"uT{s}", tag=f"uT{s}")
            uT.append(u)

        NG = NMT // 2  # 16 m-tile pairs
        for g in range(NG):
            for s in range(NSB):
                stp = st_pool.tile([128, 1024], F32, tag="st")
                nc.tensor.matmul(stp[:, 0:SBW],
                                 kT_sb[0:33, ts(g, 128)],
                                 qT_sb[0:33, ts(s, SBW)],
                                 tile_position=(0, 0))
                nc.tensor.matmul(stp[:, SBW:1024],
                                 kT_sb[64:97, ts(g, 128)],
                                 qT_sb[64:97, ts(s, SBW)],
                                 tile_position=(64, 0))
                ex = expp.tile([128, 1024], F32R)
                nc.scalar.activation(ex[:], stp[:],
                                     mybir.ActivationFunctionType.Exp)
                nc.tensor.matmul(uT[s][:],
                                 v_sb[:, 33 * g:33 * g + 33],
                                 ex[:, 0:SBW],
                                 start=(g == 0), stop=False,
                                 skip_group_check=True)
                nc.tensor.matmul(uT[s][:],
                                 v_sb[:, 33 * (g + 16):33 * (g + 16) + 33],
                                 ex[:, SBW:1024],
                                 start=False, stop=(g == NG - 1),
                                 skip_group_check=True)

        # ---- output projection (O1 col 256 = row-sums r) + normalize ----
        # wo_aug2: rows 0:32 = gamma*Wo with a zero col 256; row 32 =
        # zeros except [32, 256] = 1, so O1[:, 256] = r. out = O1*1/r + xr'
        # (bo and bv folds are host-added to xr').
        for s in range(NSB):
            usb = smallp.tile([33, SBW], F32R, tag="usb")
            nc.vector.tensor_copy(usb[:], uT[s][:])
            for qb in range(SBW // 128):
                o_ps = st_pool.tile([128, C + 2], F32, tag="st")
                nc.tensor.matmul(o_ps[:], usb[:, ts(qb, 128)], wo_sb[:])
                recip = smallp.tile([128, 1], F32, tag="recip")
                nc.vector.reciprocal(recip[:], o_ps[:, C:C + 1])
                qi = s * (SBW // 128) + qb
                ost = outp.tile([128, C], F32)
                nc.vector.scalar_tensor_tensor(
                    ost[:], o_ps[:, 0:C], recip[:], xr_sb[:, ts(qi, C)],
                    op0=mybir.AluOpType.mult, op1=mybir.AluOpType.add,
                )
                nc.sync.dma_start(out_d[ts(qi, 128), :], ost[:])

    nc.compile()
    return nc


_NC_CACHE = None


def _get_nc():
    global _NC_CACHE
    if _NC_CACHE is None:
        _NC_CACHE = build_graph()
    return _NC_CACHE


def make_in_maps(x, Wk, bk, Wq, bq, Wv, bv, Wo, bo, gamma):
    """Host-side sharding + exact bias/gamma folding."""
    f32 = np.float32
    xf = np.ascontiguousarray(x, dtype=f32).reshape(B, N, C)
    Wk = np.asarray(Wk, dtype=f32)
    Wq = np.asarray(Wq, dtype=f32)
    Wv = np.asarray(Wv, dtype=f32)
    Wo = np.asarray(Wo, dtype=f32)
    bk = np.asarray(bk, dtype=f32)
    bq = np.asarray(bq, dtype=f32)
    bv = np.asarray(bv, dtype=f32)
    bo = np.asarray(bo, dtype=f32)
    g = np.asarray(gamma, dtype=f32)[0]

    wk_aug = np.concatenate([Wk, (Wk @ bq)[:, None]], axis=1)  # [C, 33]
    wo_aug = np.zeros((33, C + 2), dtype=f32)
    wo_aug[0:32, 0:C] = g * Wo
    wo_aug[32, C] = 1.0
    xr_bias = (g * (bv @ Wo + bo)).astype(f32)  # folded into the residual

    in_maps = []
    for i in range(NCORES):
        b, h = divmod(i, 2)
        own = xf[b, h * QSH:(h + 1) * QSH]
        other = xf[b, (1 - h) * QSH:(2 - h) * QSH]
        xT = np.ascontiguousarray(
            np.concatenate([own, other], axis=0).T
        )  # [C, N], own-half columns first
        in_maps.append({
            "xT": xT,
            "xr": np.ascontiguousarray(own + xr_bias),
            "wk": np.ascontiguousarray(wk_aug),
            "wq": Wq,
            "wv": Wv,
            "wo": np.ascontiguousarray(wo_aug),
        })
    return in_maps


def gather_out(results, x_dtype):
    out = np.empty((B, N, C), dtype=np.float32)
    for i in range(NCORES):
        b, h = divmod(i, 2)
        out[b, h * QSH:(h + 1) * QSH] = results[i]["out"]
    return out.reshape(B, HH, WW, C).astype(x_dtype, copy=False)


def kernel(x, Wk, bk, Wq, bq, Wv, bv, Wo, bo, gamma, **run_kwargs):
    nc = _get_nc()
    in_maps = make_in_maps(x, Wk, bk, Wq, bq, Wv, bv, Wo, bo, gamma)
    res = bass_utils.run_bass_kernel_spmd(
        nc, in_maps, core_ids=list(range(NCORES)), **run_kwargs
    )
    out = gather_out(res.results, np.asarray(x).dtype)
    if run_kwargs:
        return out, res
    return out
